# revision 18
# baseline (speedup 1.0000x reference)
"""Trainium2 Bass kernel for nn_AdditiveIntervention.

Reference computation (B=512, N=1024, D=FUSE=1024, A=256):
    q = fuse_rep @ Wq                               # [B, A]
    k = confounder_set @ Wk                         # [N, A]
    scores[b,n] = sum_a wt[a] * tanh(q[b,a]+k[n,a]) # [B, N]
    attn = softmax(scores, axis=1)
    out = (attn * probs) @ confounder_set           # [B, D]

Sharding: data-parallel over B across 8 NeuronCores (64 rows each); the
confounder set and weights are replicated. Host-side prep: transposes,
partition-major pre-tiling (so every DMA is contiguous per partition),
f32->bf16 casts of PE-facing operands, probs folded into the final
confounder operand, and the one-hot wt matrices for the score reduction.

Per-core device algorithm (a on partitions, 2 half-tiles of 128):
    qT[a, b]  = Wq[:, a].T @ fuse_rep_local.T      (PE bf16, PSUM accum)
    kT[a, n]  = Wk[:, a].T @ confT                 (PE bf16) -> f32 SBUF
    for each group of G batch rows:
        DVE: add[a, n] = kT[a, n] + qT[a, b]   (f32 tensor_scalar per row)
        ACT: tanh over the whole [128, G*1024] buffer -> bf16 out
        PE : scores[b, n-chunk] += onehot_b(wt).T @ tanh_tile  (bf16)
             (lhsT one-hot column b carrying wt -> accumulates row b in PSUM)
    softmax along free dim on [64, 1024] scores (DVE max, ACT exp+accum sum)
    wT via PE transpose (f32) -> bf16; out = wT.T @ (probs*conf) (PE bf16);
    final scale by 1/sumexp fused into the PSUM->SBUF copy (ACT scale).

Engine notes (measured on HW):
  - The ScalarE tanh stream is the roofline: 16 instrs x (224+8192)cyc
    @1.2GHz ~= 112.3us per core. Everything else hides behind it.
  - fp32 matmuls lower to 2 PE passes; all big matmuls run bf16 (1 pass).
  - GpSimd COMPUTE ops poison DVE throughput ~20x via SBUF port sharing;
    gpsimd only issues DMA here.
  - Dummy matmuls warm the PE HAM clock-gate during the DMA lead-in and
    keep it from re-throttling between per-group matmul bursts.
"""

import numpy as np

from concourse import bacc, bass, tile
import concourse.mybir as mybir
from concourse.bass_utils import run_bass_kernel_spmd

F32 = mybir.dt.float32
BF16 = mybir.dt.bfloat16
AF = mybir.ActivationFunctionType

B, N, D, FUSE, A = 512, 1024, 1024, 1024, 256
M = 8            # cores
BL = B // M      # 64 local batch rows per core
NH = A // 128    # 2 a-half tiles
G = 8            # batch rows per tanh batch
NCHUNK = 512     # psum-bank-sized matmul chunk
KT_F = FUSE // 128
NT = N // 128


def build_kernel(g: int = G):
    nc = bacc.Bacc("TRN2", target_bir_lowering=False, debug=False)

    conf_pb = nc.dram_tensor("conf_pb", [128, NT, D], BF16, kind="ExternalInput")
    confT = nc.dram_tensor("confT", [128, KT_F, N], BF16, kind="ExternalInput")
    frT = nc.dram_tensor("frT", [128, KT_F, BL], BF16, kind="ExternalInput")
    Wq = nc.dram_tensor("Wq", [128, KT_F, A], BF16, kind="ExternalInput")
    Wk = nc.dram_tensor("Wk", [128, KT_F, A], BF16, kind="ExternalInput")
    onehot_d = nc.dram_tensor("onehot", [128, NH * BL * BL], BF16, kind="ExternalInput")
    ident_d = nc.dram_tensor("ident", [BL, BL], BF16, kind="ExternalInput")
    out = nc.dram_tensor("out", [BL, D], F32, kind="ExternalOutput")

    ngroups = BL // g

    with tile.TileContext(nc) as tc:
        with (
            tc.tile_pool(name="persist", bufs=1) as pp,
            tc.tile_pool(name="scoreps", bufs=1, space="PSUM") as scorepool,
        ):
            conf_sb = pp.tile([128, NT, D], BF16)
            kT_t = {
                (h, c): pp.tile([128, NCHUNK], F32, name=f"kT_{h}_{c}")
                for h in range(NH)
                for c in range(N // NCHUNK)
            }
            qT_sb = pp.tile([128, NH, BL], F32)
            onehot = pp.tile([128, NH, BL, BL], BF16)
            act_warm = pp.tile([128, 16], F32)
            identity64 = pp.tile([BL, BL], BF16)
            warm_lhs = pp.tile([128, 64], BF16)
            warm_rhs = pp.tile([128, NCHUNK], BF16)

            scores_ps = [
                scorepool.tile([BL, NCHUNK], F32, tag=f"sc{c}", name=f"scores_ps{c}")
                for c in range(N // NCHUNK)
            ]
            warm_ps = scorepool.tile([BL, NCHUNK], F32, tag="warm")
            

            # PE HAM warm-up + ACT table preload, overlapping the DMA lead-in
            nc.vector.memset(act_warm[:], 0.0)
            nc.scalar.activation(act_warm[:], act_warm[:], AF.Tanh)
            nc.vector.memset(warm_lhs[:], 0.0)
            nc.vector.memset(warm_rhs[:], 0.0)

            # ---------------- setup ----------------
            with (
                tc.tile_pool(name="setup", bufs=1) as sp,
                tc.tile_pool(name="setps", bufs=2, space="PSUM") as setps,
            ):
                for _ in range(8):
                    nc.tensor.matmul(
                        warm_ps[:], warm_lhs[:], warm_rhs[:], start=True, stop=True
                    )
                confT_a = sp.tile([128, KT_F // 2, N], BF16)
                confT_b = sp.tile([128, KT_F // 2, N], BF16)
                Wq_sb = sp.tile([128, KT_F, A], BF16)
                Wk_sb = sp.tile([128, KT_F, A], BF16, name="Wk_sb")
                frT_sb = sp.tile([128, KT_F, BL], BF16)

                nc.sync.dma_start(confT_a[:], confT[:, 0 : KT_F // 2, :])
                nc.sync.dma_start(Wk_sb[:], Wk[:])
                nc.sync.dma_start(confT_b[:], confT[:, KT_F // 2 : KT_F, :])
                nc.sync.dma_start(Wq_sb[:], Wq[:])
                nc.sync.dma_start(frT_sb[:], frT[:])
                nc.sync.dma_start(onehot[:], onehot_d[:])
                nc.sync.dma_start(conf_sb[:], conf_pb[:])
                nc.sync.dma_start(identity64[:], ident_d[:])

                def emit_k(h, c):
                    k_ps = setps.tile([128, NCHUNK], F32, tag="kps", name="k_ps")
                    for kt in range(KT_F):
                        nc.tensor.matmul(
                            k_ps[:],
                            Wk_sb[:, kt, h * 128 : (h + 1) * 128],
                            (confT_a if kt < KT_F // 2 else confT_b)[
                                :, kt % (KT_F // 2), c * NCHUNK : (c + 1) * NCHUNK
                            ],
                            start=(kt == 0),
                            stop=(kt == KT_F - 1),
                        )
                    nc.scalar.copy(kT_t[(h, c)][:], k_ps[:])

                def emit_q(h):
                    q_ps = setps.tile([128, BL], F32, tag="qps", name="q_ps")
                    for kt in range(KT_F):
                        nc.tensor.matmul(
                            q_ps[:],
                            Wq_sb[:, kt, h * 128 : (h + 1) * 128],
                            frT_sb[:, kt, :],
                            start=(kt == 0),
                            stop=(kt == KT_F - 1),
                        )
                    nc.scalar.copy(qT_sb[:, h, :], q_ps[:])

                emit_k(0, 0)
                emit_q(0)
                emit_q(1)
                emit_k(0, 1)
                emit_k(1, 0)
                emit_k(1, 1)

            # ---------------- main loop ----------------
            groups = []
            for h in range(NH):
                for gi in range(ngroups):
                    b0 = gi * g
                    if (h == NH - 1 and gi == ngroups - 1) or (
                        h == 0 and gi == 0
                    ):
                        groups.append((h, b0, g // 2))
                        groups.append((h, b0 + g // 2, g // 2))
                    else:
                        groups.append((h, b0, g))
            with tc.tile_pool(name="fusep", bufs=2) as fp:
                for h, b0, gs in groups:
                    buf = fp.tile([128, g, N], F32, tag="fuse")
                    bufb = fp.tile([128, g, N], BF16, tag="fuseb")
                    for c in range(N // NCHUNK):
                        for j in range(gs):
                            bb = b0 + j
                            nc.vector.tensor_scalar_add(
                                buf[:, j, c * NCHUNK : (c + 1) * NCHUNK],
                                kT_t[(h, c)][:],
                                qT_sb[:, h, bb : bb + 1],
                            )
                    nc.scalar.activation(
                        bufb[:, 0:gs, :], buf[:, 0:gs, :], AF.Tanh
                    )
                    for j in range(gs):
                        bb = b0 + j
                        for c in range(N // NCHUNK):
                            nc.tensor.matmul(
                                scores_ps[c][:],
                                onehot[:, h, bb, :],
                                bufb[:, j, c * NCHUNK : (c + 1) * NCHUNK],
                                start=(h == 0 and bb == 0),
                                stop=(h == NH - 1 and bb == BL - 1),
                            )

            # ---------------- softmax + weighted sum ----------------
            with (
                tc.tile_pool(name="fin", bufs=1) as fpool,
                tc.tile_pool(name="finps", bufs=2, space="PSUM") as finps,
            ):
                for _ in range(8):
                    nc.tensor.matmul(
                        warm_ps[:], warm_lhs[:], warm_rhs[:],
                        start=True, stop=True,
                    )
                negmx_c = fpool.tile([BL, 2], F32)
                negmx = fpool.tile([BL, 1], F32)
                for c in range(N // NCHUNK):
                    nc.vector.tensor_reduce(
                        negmx_c[:, c : c + 1],
                        scores_ps[c][:],
                        mybir.AxisListType.X,
                        mybir.AluOpType.max,
                        negate=True,
                    )
                nc.vector.tensor_reduce(
                    negmx[:], negmx_c[:], mybir.AxisListType.X, mybir.AluOpType.min
                )
                wexp = fpool.tile([BL, N], BF16)
                sums_c = fpool.tile([BL, 2], F32)
                for c in range(N // NCHUNK):
                    nc.scalar.activation(
                        wexp[:, c * NCHUNK : (c + 1) * NCHUNK],
                        scores_ps[c][:],
                        AF.Exp,
                        bias=negmx[:],
                        accum_out=sums_c[:, c : c + 1],
                    )
                sums = fpool.tile([BL, 1], F32)
                nc.vector.tensor_reduce(
                    sums[:], sums_c[:], mybir.AxisListType.X, mybir.AluOpType.add
                )
                recip = fpool.tile([BL, 1], F32)
                nc.vector.reciprocal(recip[:], sums[:])

                wT = fpool.tile([128, NT, BL], BF16)
                for t in range(NT):
                    tr_ps = finps.tile([128, BL], BF16, tag="trps")
                    nc.tensor.transpose(
                        tr_ps[:], wexp[:, t * 128 : (t + 1) * 128], identity64[:]
                    )
                    nc.vector.tensor_copy(wT[:, t, :], tr_ps[:])

                out_sb = fpool.tile([BL, D], F32)
                for dc in range(D // NCHUNK):
                    f_ps = finps.tile([BL, NCHUNK], F32, tag="fps")
                    for t in range(NT):
                        nc.tensor.matmul(
                            f_ps[:],
                            wT[:, t, :],
                            conf_sb[:, t, dc * NCHUNK : (dc + 1) * NCHUNK],
                            start=(t == 0),
                            stop=(t == NT - 1),
                        )
                    nc.scalar.activation(
                        out_sb[:, dc * NCHUNK : (dc + 1) * NCHUNK],
                        f_ps[:],
                        AF.Copy,
                        scale=recip[:],
                    )
                    nc.sync.dma_start(
                        out[:, dc * NCHUNK : (dc + 1) * NCHUNK],
                        out_sb[:, dc * NCHUNK : (dc + 1) * NCHUNK],
                    )

    nc.compile()
    return nc


_NC_CACHE = {}


def _get_nc(g: int = G):
    if g not in _NC_CACHE:
        _NC_CACHE[g] = build_kernel(g)
    return _NC_CACHE[g]


def _tile128(x):
    """[t*128, C] row-major -> [128, t, C] partition-major (contiguous DMA)."""
    t = x.shape[0] // 128
    return np.ascontiguousarray(
        x.reshape(t, 128, x.shape[1]).transpose(1, 0, 2)
    )


def _make_in_maps(inputs):
    import ml_dtypes

    bf = ml_dtypes.bfloat16
    conf = np.asarray(inputs["confounder_set"], np.float32)      # [N, D]
    fr = np.asarray(inputs["fuse_rep"], np.float32)              # [B, FUSE]
    probs = np.asarray(inputs["probabilities"], np.float32).reshape(N)
    Wq = np.asarray(inputs["Wq"], np.float32)
    Wk = np.asarray(inputs["Wk"], np.float32)
    wt = np.asarray(inputs["wt"], np.float32)

    conf_pb = _tile128((probs[:, None] * conf).astype(bf))
    confT = _tile128(conf.T.astype(bf))
    frT_full = fr.T.astype(bf)                                   # [FUSE, B]
    Wq_b = _tile128(Wq.astype(bf))
    Wk_b = _tile128(Wk.astype(bf))

    # onehot[p, h, b, m] = wt[h*128+p] * (b == m), flattened to [128, NH*BL*BL]
    wtT = wt.reshape(NH, 128).T                                  # [128, NH]
    onehot_f = np.zeros((128, NH, BL, BL), dtype=np.float32)
    idx = np.arange(BL)
    onehot_f[:, :, idx, idx] = wtT[:, :, None]
    onehot = np.ascontiguousarray(onehot_f.astype(bf).reshape(128, NH * BL * BL))

    ident = np.eye(BL, dtype=bf)

    in_maps = []
    for c in range(M):
        in_maps.append(
            {
                "conf_pb": conf_pb,
                "confT": confT,
                "frT": _tile128(
                    np.ascontiguousarray(frT_full[:, c * BL : (c + 1) * BL])
                ),
                "Wq": Wq_b,
                "Wk": Wk_b,
                "onehot": onehot,
                "ident": ident,
            }
        )
    return in_maps


def _run(inputs, trace: bool = False):
    nc = _get_nc()
    in_maps = _make_in_maps(inputs)
    res = run_bass_kernel_spmd(nc, in_maps, core_ids=list(range(M)), trace=trace)
    out_full = np.concatenate(
        [res.results[i]["out"] for i in range(M)], axis=0
    ).astype(np.float32)
    return out_full, res


def kernel(**inputs) -> np.ndarray:
    out, _ = _run(inputs)
    return out
